# revision 35
# baseline (speedup 1.0000x reference)
"""NewtonNet embedding kernel for 8 TRN2 NeuronCores (Bass/Tile) — v2.

Strategy (graph/data parallel on edges, per the sharding hint):
  - Edges (sorted by src) are sharded 8 ways at segment boundaries, so each
    core's segment sums are fully local. Each core owns the contiguous node
    range its edges cover.
  - Within a core, owned nodes are sorted by degree; edges are laid out
    host-side so that the edge for node-slot `128*t + p` (round r) sits at
    partition p of edge block (tile t, round r). Segment sums then become
    plain per-partition accumulates, and the src-side gather is the identity.
  - dst-side gather: per layer, each core's a-MLP output `ai` is AllGathered
    (bf16) into a shared DRAM buffer; `dma_gather` pulls the dst rows with
    host-pretranslated (owner, slot) int16 indices.
  - Layer 0 needs no collective: xi0 = onehot(species) @ species_W is
    computed locally for ALL nodes (species is a replicated input), so each
    core materializes the full ai0 table itself.

v2 performance structure (vs v1):
  - All matmuls in bf16 (4x PE throughput vs fp32 4-cycle mode).
  - mij -> mijT via the DMA crossbar transpose (dma_start_transpose), off
    the PE/ACT engines.
  - phi (l>0) accumulated in PSUM by pre-scaling h1_r with sw along the
    free dim, so the r-mm2 itself does the segment sum.
  - fi updates as 4x-mode fp16 STTs on DVE (all-SBUF 16-bit operands).
  - F-scalar bias folded host-side into dirsw*F_b2 so Fv is one STT.
  - fp16 accumulators (fi/di/phi/scal/xiacc) for 2x/4x DVE modes.
"""

import sys

import numpy as np

sys.path.insert(0, "/opt/trn_rl_repo")

N_CORES = 8
DIM = 128
NLAYERS = 3
NBASIS = 16
CUTOFF = 5.0
CHUNK_BLOCKS = 8  # edge blocks per dma_gather chunk (1024 idxs)

# ---------------------------------------------------------------------------
# Host-side preprocessing
# ---------------------------------------------------------------------------


def host_prep(inputs):
    species = np.asarray(inputs["species"])
    src = np.asarray(inputs["edge_src"])
    dst = np.asarray(inputs["edge_dst"])
    d = np.asarray(inputs["distances"]).astype(np.float32)
    vec = np.asarray(inputs["vec"]).astype(np.float32)
    sw = np.asarray(inputs["switch"]).astype(np.float32)
    E = src.shape[0]
    N = species.shape[0]

    cuts = [0]
    for c in range(1, N_CORES):
        k = c * E // N_CORES
        while k < E and src[k] == src[k - 1]:
            k += 1
        cuts.append(k)
    cuts.append(E)
    nb = [0]
    for c in range(1, N_CORES):
        nb.append(int(src[cuts[c]]) if cuts[c] < E else N)
    nb.append(N)

    deg = np.bincount(src, minlength=N)
    cores = []
    for c in range(N_CORES):
        lo, hi = nb[c], nb[c + 1]
        nodes = np.arange(lo, hi)
        order = np.argsort(-deg[lo:hi], kind="stable")
        perm = nodes[order]
        cores.append(dict(perm=perm, deg=deg[perm]))

    NT = max((len(c["perm"]) + 127) // 128 for c in cores)
    n_pad = NT * 128

    R = []
    for t in range(NT):
        r = 0
        for c in cores:
            dg = c["deg"]
            if 128 * t < len(dg):
                r = max(r, int(dg[128 * t]))
        R.append(max(r, 1))
    B = sum(R)
    E_dev = 128 * B

    owner = np.zeros(N, np.int32)
    slot = np.zeros(N, np.int32)
    for ci, c in enumerate(cores):
        owner[c["perm"]] = ci
        slot[c["perm"]] = np.arange(len(c["perm"]))

    # species per global slot (absolute core-major order), same for all cores
    spec_glob = np.zeros(N_CORES * n_pad, np.float32)
    for ci, c in enumerate(cores):
        spec_glob[ci * n_pad : ci * n_pad + len(c["perm"])] = species[c["perm"]]

    centers = np.linspace(0.0, CUTOFF, NBASIS).astype(np.float32)
    eta = np.float32((NBASIS / CUTOFF) ** 2)
    estart = np.zeros(N + 1, np.int64)
    np.cumsum(deg, out=estart[1:])

    bF = [float(np.asarray(inputs["F_b2"])[l, 0]) for l in range(NLAYERS)]

    per_core = []
    for ci, c in enumerate(cores):
        perm, dgs = c["perm"], c["deg"]
        n_own = len(perm)
        eid = np.full(E_dev, -1, np.int64)
        off = 0
        for t in range(NT):
            s0 = 128 * t
            s1 = min(s0 + 128, n_own)
            if s1 > s0:
                pp = np.arange(s0, s1)
                dg_t = dgs[pp]
                base = estart[perm[pp]]
                for r in range(R[t]):
                    b = off + r
                    m = r < dg_t
                    idx = 128 * b + (pp - s0)
                    eid[idx[m]] = base[m] + r
            off += R[t]
        valid = eid >= 0
        ge = np.where(valid, eid, 0)

        de = d[ge]
        swe = np.where(valid, sw[ge], 0.0).astype(np.float32)
        rbsw = np.zeros((17, E_dev), np.float32)
        rbsw[:16] = np.exp(-eta * (de[None, :] - centers[:, None]) ** 2) * swe[None, :]
        rbsw[16] = swe

        dire = vec[ge] / de[:, None] * swe[:, None]

        dsti = (owner[dst[ge]].astype(np.int64) * n_pad + slot[dst[ge]]).astype(
            np.int16
        )
        dsti[~valid] = 0

        sw_dev = swe.reshape(B, 128)
        swsum = np.zeros((128, NT), np.float32)
        o = 0
        for t in range(NT):
            swsum[:, t] = sw_dev[o : o + R[t]].sum(axis=0)
            o += R[t]
        swb = np.ascontiguousarray(np.broadcast_to(swe[None, :], (128, E_dev)))

        # ddiag[(e), (b,k,j)] = dirsw_k[e] * delta(e==j): per-block diagonal
        # selection matrices so the PE does fi_k += diag(dirsw_k).T @ g
        import ml_dtypes

        dd = np.zeros((B, 3, 128, 128), ml_dtypes.bfloat16)
        ar = np.arange(128)
        dd[:, :, ar, ar] = dire.reshape(B, 128, 3).transpose(0, 2, 1).astype(
            ml_dtypes.bfloat16
        )
        ddiag = np.ascontiguousarray(
            dd.transpose(2, 0, 1, 3).reshape(128, B * 3 * 128)
        )
        del dd
        dsti_w = np.tile(dsti.reshape(-1, 16).T, (8, 1))  # replicated for 8 Q7 cores

        spec_own = np.zeros(n_pad, np.float32)
        spec_own[:n_own] = species[perm]
        specb_own = np.ascontiguousarray(
            np.broadcast_to(spec_own[None, :], (128, n_pad))
        )

        per_core.append(
            dict(
                rbsw=rbsw.astype(np.float32),
                ddiag=ddiag,
                swb=swb,
                swsum=swsum,
                dsti_w=dsti_w,
                specb_own=specb_own,
                perm=perm,
                n_own=n_own,
            )
        )

    specb_glob = np.ascontiguousarray(
        np.broadcast_to(spec_glob[None, :], (128, N_CORES * n_pad))
    )
    bFcols = np.broadcast_to(
        np.asarray(bF, np.float32)[None, :], (128, NLAYERS)
    ).copy()
    meta = dict(NT=NT, n_pad=n_pad, R=R, B=B, E_dev=E_dev, specb_glob=specb_glob,
                bFcols=bFcols)
    return meta, per_core


def pack_weights(inputs):
    """Pack all weight matrices into one [NW,128,128] tensor + bias tensors."""
    W = {k: np.asarray(v).astype(np.float32) for k, v in inputs.items()}
    mats = []
    widx = {}

    def add(name, m):
        widx[name] = len(mats)
        p = np.zeros((128, 128), np.float32)
        p[: m.shape[0], : m.shape[1]] = m
        mats.append(p)

    for l in range(NLAYERS):
        for nm in ["a", "F", "f", "r", "R", "u"]:
            add(f"{nm}_W1_{l}", W[f"{nm}_W1"][l])
            add(f"{nm}_W2_{l}", W[f"{nm}_W2"][l])
        add(
            f"radial_Wb_{l}",
            np.concatenate([W["radial_W"][l], W["radial_b"][l][None]], 0),
        )
    spw = np.zeros((128, 128), np.float32)
    spw[:100] = W["species_W"]
    add("spw", spw)
    wts = np.stack(mats)  # [NW,128,128]

    # bias columns [128, 19]: per layer a,F,f,r,R,u b1 (6 cols); col 18 = species_b
    bcols = np.zeros((128, 6 * NLAYERS + 1), np.float32)
    for l in range(NLAYERS):
        for j, nm in enumerate(["a", "F", "f", "r", "R", "u"]):
            bcols[:, 6 * l + j] = W[f"{nm}_b1"][l]
    bcols[:, 6 * NLAYERS] = W["species_b"]

    # broadcast b2 tiles [NB,128,128]: per-layer a,f,r,R,u
    bidx = {}
    bl = []
    for nm in ["a", "f", "r", "R", "u"]:
        for l in range(NLAYERS):
            bidx[f"{nm}_{l}"] = len(bl)
            bl.append(np.tile(W[f"{nm}_b2"][l][None], (128, 1)))
    bbc = np.stack(bl)
    return wts, widx, bcols, bbc, bidx


# ---------------------------------------------------------------------------
# Device kernel builder
# ---------------------------------------------------------------------------


def build_nc(meta, widx, bidx, NW, NB_bbc, no_cc=False, no_gather=False,
             direct_shared=True, nlayers=NLAYERS):
    import concourse.bass as bass
    import concourse.bacc as bacc
    import concourse.mybir as mybir
    import concourse.tile as tile
    from concourse.masks import make_identity

    NT, n_pad, R, B, E_dev = meta["NT"], meta["n_pad"], meta["R"], meta["B"], meta["E_dev"]
    NG = N_CORES * n_pad  # global padded node count
    f32 = mybir.dt.float32
    f16 = mybir.dt.float16
    bf16 = mybir.dt.bfloat16
    i16 = mybir.dt.int16
    Alu = mybir.AluOpType
    Act = mybir.ActivationFunctionType

    nc = bacc.Bacc("TRN2", target_bir_lowering=False, debug=False, num_devices=N_CORES)

    # ---- I/O ----
    wts_d = nc.dram_tensor("wts", [NW, 128, 128], bf16, kind="ExternalInput")
    bcols_d = nc.dram_tensor("bcols", [128, 6 * NLAYERS + 1], f32, kind="ExternalInput")
    bbc_d = nc.dram_tensor("bbc", [NB_bbc, 128, 128], bf16, kind="ExternalInput")
    rbsw_d = nc.dram_tensor("rbsw", [17, E_dev], bf16, kind="ExternalInput")
    dsti_d = nc.dram_tensor("dsti", [128, E_dev // 16], i16, kind="ExternalInput")
    ddiag_d = nc.dram_tensor("ddiag", [128, B * 3 * 128], bf16, kind="ExternalInput")
    bFcols_d = nc.dram_tensor("bFcols", [128, NLAYERS], f32, kind="ExternalInput")
    swb_d = nc.dram_tensor("swb", [128, E_dev], bf16, kind="ExternalInput")
    swsum_d = nc.dram_tensor("swsum", [128, NT], f32, kind="ExternalInput")
    specown_d = nc.dram_tensor("specb_own", [128, n_pad], bf16, kind="ExternalInput")
    specglob_d = nc.dram_tensor("specb_glob", [128, NG], bf16, kind="ExternalInput")
    iota_d = nc.dram_tensor("iota128", [128, 1], f32, kind="ExternalInput")
    xi_out_d = nc.dram_tensor("xi_out", [n_pad, 128], f32, kind="ExternalOutput")

    ai_bounce = nc.dram_tensor("ai_bounce", [n_pad, 128], bf16)
    ai_all_sh = nc.dram_tensor("ai_all_sh", [NG, 128], bf16, addr_space="Shared")
    ai_all = nc.dram_tensor("ai_all", [NG, 128], bf16)

    # ---- persistent SBUF ----
    wts_sb = nc.alloc_sbuf_tensor("wts_sb", [128, NW * 128], bf16)
    bcols_sb = nc.alloc_sbuf_tensor("bcols_sb", [128, 6 * NLAYERS + 1], f32)
    bbc_sb = nc.alloc_sbuf_tensor("bbc_sb", [128, NB_bbc * 128], bf16)
    bFcols_sb = nc.alloc_sbuf_tensor("bFcols_sb", [128, NLAYERS], f32)
    swsum_sb = nc.alloc_sbuf_tensor("swsum_sb", [128, NT], f32)
    dsti_sb = nc.alloc_sbuf_tensor("dsti_sb", [128, E_dev // 16], i16)
    specown_sb = nc.alloc_sbuf_tensor("specown_sb", [128, n_pad], bf16)
    specglob_sb = nc.alloc_sbuf_tensor("specglob_sb", [128, NG], bf16)
    iota_sb = nc.alloc_sbuf_tensor("iota_sb", [128, 1], f32)
    identf_sb = nc.alloc_sbuf_tensor("identf_sb", [128, 128], f32)
    identb_sb = nc.alloc_sbuf_tensor("identb_sb", [128, 128], bf16)

    xi_sb = nc.alloc_sbuf_tensor("xi_sb", [128, NT * 128], f32)
    xiT_sb = nc.alloc_sbuf_tensor("xiT_sb", [128, n_pad], bf16)
    ai_sb = nc.alloc_sbuf_tensor("ai_sb", [128, NT * 128], bf16)
    fi_sb = nc.alloc_sbuf_tensor("fi_sb", [128, NT * 3 * 128], f16)
    di_sb = nc.alloc_sbuf_tensor("di_sb", [128, NT * 3 * 128], f16)
    phi_sb = nc.alloc_sbuf_tensor("phi_sb", [128, NT * 128], f16)
    Rout_sb = nc.alloc_sbuf_tensor("Rout_sb", [128, NT * 128], f16)
    uout_sb = nc.alloc_sbuf_tensor("uout_sb", [128, NT * 128], f16)
    scal_sb = nc.alloc_sbuf_tensor("scal_sb", [128, NT * 128], f16)
    tmpB_sb = nc.alloc_sbuf_tensor("tmpB_sb", [128, NT * 128], f16)

    def W(name):
        m = widx[name]
        return wts_sb[:, 128 * m : 128 * (m + 1)]

    def Bb(name):
        m = bidx[name]
        return bbc_sb[:, 128 * m : 128 * (m + 1)]

    def ap3(ap2d, base, gw):
        """[128, gw, 128] view of contiguous cols [base, base+gw*128)."""
        a = ap2d[:, base : base + gw * 128]
        return bass.AP(
            tensor=a.tensor, offset=a.offset, ap=[a.ap[0], [128, gw], [1, 128]]
        )

    def rep3(ap2d, base, gw):
        """[128, gw, 128] broadcast of cols [base, base+128) repeated gw times."""
        a = ap2d[:, base : base + 128]
        return bass.AP(tensor=a.tensor, offset=a.offset, ap=[a.ap[0], [0, gw], [1, 128]])

    # tile index of each block, block ranges per tile
    tile_off = []
    o = 0
    for t in range(NT):
        tile_off.append(o)
        o += R[t]

    b1col = lambda l, nm: bcols_sb[:, 6 * l + ["a", "F", "f", "r", "R", "u"].index(nm)
                                   : 6 * l + ["a", "F", "f", "r", "R", "u"].index(nm) + 1]

    with tile.TileContext(nc) as tc:
        with (
            tc.tile_pool(name="psA", bufs=2, space="PSUM") as psA,  # dij / onehot / transposes
            tc.tile_pool(name="psC", bufs=2, space="PSUM") as psC,  # mm1 h1
            tc.tile_pool(name="psD", bufs=2, space="PSUM") as psD,  # mm2 outs
            tc.tile_pool(name="psPhi", bufs=1, space="PSUM") as psPhi,  # phi+xiacc accum
            tc.tile_pool(name="psFi", bufs=1, space="PSUM") as psFi,  # fi accum
            tc.tile_pool(name="sb", bufs=3) as sbp,  # working sbuf
            tc.tile_pool(name="sbh", bufs=6) as sbh,  # h1 tiles
            tc.tile_pool(name="sbg", bufs=2) as sbg,  # gather chunks
        ):
            # ---- load resident data ----
            nc.sync.dma_start(
                wts_sb[:].rearrange("p (m n) -> p m n", m=NW),
                wts_d[:].rearrange("m p n -> p m n"),
            )
            nc.sync.dma_start(bcols_sb[:], bcols_d[:])
            nc.sync.dma_start(
                bbc_sb[:].rearrange("p (m n) -> p m n", m=NB_bbc),
                bbc_d[:].rearrange("m p n -> p m n"),
            )
            nc.sync.dma_start(bFcols_sb[:], bFcols_d[:])
            nc.sync.dma_start(swsum_sb[:], swsum_d[:])
            nc.sync.dma_start(dsti_sb[:], dsti_d[:])
            nc.sync.dma_start(specown_sb[:], specown_d[:])
            nc.sync.dma_start(specglob_sb[:], specglob_d[:])
            nc.sync.dma_start(iota_sb[:], iota_d[:])
            make_identity(nc, identf_sb[:])
            make_identity(nc, identb_sb[:])
            nc.vector.memset(fi_sb[:], 0.0)

            def onehot_embed(specb, c0, w, out_sb_slice):
                """out = (species_W.T @ onehot(spec[c0:c0+w])) + species_b, bf16 fm."""
                oh = sbp.tile([128, 512], bf16, tag="oh")
                nc.vector.tensor_scalar(
                    out=oh[:, :w],
                    in0=specb[:, c0 : c0 + w],
                    scalar1=iota_sb[:, 0:1],
                    scalar2=None,
                    op0=Alu.is_equal,
                )
                ps = psA.tile([128, 512], f32, tag="ps512a")
                nc.tensor.matmul(ps[:, :w], lhsT=W("spw"), rhs=oh[:, :w],
                                 start=True, stop=True)
                nc.scalar.activation(
                    out=out_sb_slice, in_=ps[:, :w], func=Act.Identity,
                    bias=bcols_sb[:, 6 * NLAYERS : 6 * NLAYERS + 1],
                )

            def node_mlp(l, nm, src_sb, width, out_sb, b2name, dt_note=None):
                """out = silu(src.T @ W1 + b1) @ W2 + b2, node-major output."""
                W1, W2 = W(f"{nm}_W1_{l}"), W(f"{nm}_W2_{l}")
                for c0 in range(0, width, 512):
                    w = min(512, width - c0)
                    h1p = psC.tile([128, 512], f32, tag="h1")
                    nc.tensor.matmul(
                        h1p[:, :w], lhsT=W1, rhs=src_sb[:, c0 : c0 + w],
                        start=True, stop=True,
                    )
                    h1s = sbh.tile([128, 512], bf16, tag="h1s")
                    nc.scalar.activation(
                        out=h1s[:, :w], in_=h1p[:, :w], func=Act.Silu,
                        bias=b1col(l, nm),
                    )
                    mp = psD.tile([128, 512], f32, tag="mm2")
                    for j in range(w // 128):
                        nc.tensor.matmul(
                            mp[:, 128 * j : 128 * (j + 1)],
                            lhsT=h1s[:, 128 * j : 128 * (j + 1)],
                            rhs=W2, start=True, stop=True,
                        )
                    nc.vector.tensor_tensor(
                        out=ap3(out_sb, c0, w // 128),
                        in0=bass.AP(tensor=mp[:].tensor, offset=mp[:].offset,
                                    ap=[mp[:].ap[0], [128, w // 128], [1, 128]]),
                        in1=rep3(bbc_sb[:], 128 * bidx[b2name], w // 128),
                        op=Alu.add,
                    )

            # =========================== layer 0 bootstrap ===========================
            # (a) own nodes: xi0T (fm) -> xi_sb (node-major) + ai via a-MLP
            xi0T = xiT_sb  # reuse: xi0T is exactly xiT for layer 0
            for c0 in range(0, n_pad, 512):
                w = min(512, n_pad - c0)
                onehot_embed(specown_sb, c0, w, xi0T[:, c0 : c0 + w])
            # xi_sb = transpose(xi0T)
            for t0 in range(0, NT, 4):
                tn = min(4, NT - t0)
                ps = psA.tile([128, 512], bf16, tag="ps512a")
                for j in range(tn):
                    nc.tensor.transpose(
                        out=ps[:, 128 * j : 128 * (j + 1)],
                        in_=xi0T[:, 128 * (t0 + j) : 128 * (t0 + j + 1)],
                        identity=identb_sb[:],
                    )
                nc.scalar.copy(out=xi_sb[:, 128 * t0 : 128 * (t0 + tn)],
                               in_=ps[:, : 128 * tn])

            # (b) global ai0 table (absolute slot order), written to local DRAM
            for c0 in range(0, NG, 512):
                xg = sbp.tile([128, 512], bf16, tag="xg")
                onehot_embed(specglob_sb, c0, 512, xg[:])
                h1p = psC.tile([128, 512], f32, tag="h1")
                nc.tensor.matmul(h1p[:], lhsT=W("a_W1_0"), rhs=xg[:],
                                 start=True, stop=True)
                h1s = sbh.tile([128, 512], bf16, tag="h1s")
                nc.scalar.activation(out=h1s[:], in_=h1p[:], func=Act.Silu,
                                     bias=b1col(0, "a"))
                mp = psD.tile([128, 512], f32, tag="mm2")
                for j in range(4):
                    nc.tensor.matmul(
                        mp[:, 128 * j : 128 * (j + 1)],
                        lhsT=h1s[:, 128 * j : 128 * (j + 1)],
                        rhs=W("a_W2_0"), start=True, stop=True,
                    )
                aig = sbp.tile([128, 512], bf16, tag="aig")
                nc.vector.tensor_tensor(
                    out=ap3(aig[:], 0, 4),
                    in0=bass.AP(tensor=mp[:].tensor, offset=mp[:].offset,
                                ap=[mp[:].ap[0], [128, 4], [1, 128]]),
                    in1=rep3(bbc_sb[:], 128 * bidx["a_0"], 4),
                    op=Alu.add,
                )
                nc.sync.dma_start(
                    ai_all[c0 : c0 + 512, :].rearrange("(t p) c -> p t c", p=128),
                    aig[:].rearrange("p (t c) -> p t c", t=4),
                )

            # =========================== layers ===========================
            for l in range(nlayers):
                if l == 0:
                    node_mlp(0, "a", xi0T, n_pad, ai_sb, "a_0")
                else:
                    # transpose xi -> xiT (bf16)
                    for t0 in range(0, NT, 4):
                        tn = min(4, NT - t0)
                        ps = psA.tile([128, 512], f32, tag="ps512a")
                        for j in range(tn):
                            nc.tensor.transpose(
                                out=ps[:, 128 * j : 128 * (j + 1)],
                                in_=xi_sb[:, 128 * (t0 + j) : 128 * (t0 + j + 1)],
                                identity=identf_sb[:],
                            )
                        nc.scalar.copy(out=xiT_sb[:, 128 * t0 : 128 * (t0 + tn)],
                                       in_=ps[:, : 128 * tn])
                    node_mlp(l, "a", xiT_sb, n_pad, ai_sb, f"a_{l}")
                    # ai -> DRAM bounce -> AllGather
                    nc.sync.dma_start(
                        ai_bounce[:].rearrange("(t p) c -> p t c", p=128),
                        ai_sb[:].rearrange("p (t c) -> p t c", t=NT),
                    )
                    if no_cc:
                        for _ci in range(N_CORES):
                            nc.sync.dma_start(
                                ai_all[_ci * n_pad : (_ci + 1) * n_pad, :], ai_bounce[:]
                            )
                    else:
                        nc.gpsimd.collective_compute(
                            "AllGather",
                            Alu.bypass,
                            replica_groups=[list(range(N_CORES))],
                            ins=[ai_bounce.ap().opt()],
                            outs=[ai_all_sh.ap().opt()],
                        )
                        if not direct_shared:
                            nc.sync.dma_start(ai_all[:], ai_all_sh[:])

                gather_src = ai_all_sh if (direct_shared and not no_cc and l > 0) else ai_all

                # ---- edge phase ----
                chunk_tiles = {}

                def get_chunk(ci, gather_src=gather_src):
                    if ci in chunk_tiles:
                        return chunk_tiles[ci]
                    b0 = ci * CHUNK_BLOCKS
                    nb_ = min(CHUNK_BLOCKS, B - b0)
                    # dep-free bulk loads issue from the gpsimd queue (cheap
                    # issue, never behind a data-dependent wait), ahead of the
                    # gather in its FIFO
                    rb = sbg.tile([17, CHUNK_BLOCKS * 128], bf16, tag="rbsw")
                    nc.gpsimd.dma_start(
                        rb[:, : nb_ * 128], rbsw_d[:, 128 * b0 : 128 * (b0 + nb_)]
                    )
                    dd = sbg.tile([128, CHUNK_BLOCKS * 3 * 128], bf16, tag="ddiag")
                    nc.gpsimd.dma_start(
                        dd[:, : nb_ * 384],
                        ddiag_d[:, 384 * b0 : 384 * (b0 + nb_)],
                    )
                    swc = None
                    if l > 0:
                        swc = sbg.tile([128, CHUNK_BLOCKS * 128], bf16, tag="swb")
                        nc.gpsimd.dma_start(
                            swc[:, : nb_ * 128], swb_d[:, 128 * b0 : 128 * (b0 + nb_)]
                        )
                    ad = sbg.tile([128, CHUNK_BLOCKS * 128], bf16, tag="adst")
                    if no_gather:
                        nc.vector.memset(ad[:], 1.0)
                    else:
                        nc.gpsimd.dma_gather(
                            out_ap=ad[:, : nb_ * 128].rearrange("p (b c) -> p b c", b=nb_),
                            in_ap=gather_src[:],
                            idxs_ap=dsti_sb[:, 8 * b0 : 8 * (b0 + nb_)],
                            num_idxs=128 * nb_,
                            num_idxs_reg=128 * nb_,
                            elem_size=128,
                        )
                    chunk_tiles[ci] = (ad, rb, dd, swc, b0)
                    return chunk_tiles[ci]

                for t in range(NT):
                    phx = psPhi.tile([128, 256], f32, tag="phx")  # phi | xiacc
                    fi_ps = psFi.tile([128, 3 * 129], f32, tag="fia")
                    r0 = 0
                    while r0 < R[t]:
                        b0 = tile_off[t] + r0
                        gw = min(4, R[t] - r0, CHUNK_BLOCKS - b0 % CHUNK_BLOCKS)
                        ci = b0 // CHUNK_BLOCKS
                        ad, rb, dd, swc, cb0 = get_chunk(ci)
                        w = gw * 128
                        boff = (b0 - cb0) * 128
                        first = r0 == 0
                        last_g = r0 + gw == R[t]

                        dij = psA.tile([128, 512], f32, tag="ps512a")
                        for j in range(gw):
                            nc.tensor.matmul(
                                dij[:, 128 * j : 128 * (j + 1)],
                                lhsT=rb[:17, boff + 128 * j : boff + 128 * (j + 1)],
                                rhs=W(f"radial_Wb_{l}")[:17, :],
                                start=True, stop=True,
                            )
                        # mij = (dij * ad) * ai_src  (bf16, edge-major)
                        mij = sbp.tile([128, 512], bf16, tag="mij")
                        nc.vector.scalar_tensor_tensor(
                            out=ap3(mij[:], 0, gw),
                            in0=bass.AP(tensor=dij[:].tensor, offset=dij[:].offset,
                                        ap=[dij[:].ap[0], [128, gw], [1, 128]]),
                            scalar=1.0,
                            in1=ap3(ad[:], boff, gw),
                            op0=Alu.mult,
                            op1=Alu.mult,
                        )
                        nc.vector.scalar_tensor_tensor(
                            out=ap3(mij[:], 0, gw),
                            in0=ap3(mij[:], 0, gw),
                            scalar=1.0,
                            in1=rep3(ai_sb[:], 128 * t, gw),
                            op0=Alu.mult,
                            op1=Alu.mult,
                        )
                        # mij -> mijT via DMA crossbar transpose
                        mijT = sbp.tile([128, 512], bf16, tag="mijT")
                        nc.sync.dma_start_transpose(
                            mijT[:, :w].rearrange("p (j c) -> p j c", j=gw),
                            mij[:, :w],
                        )

                        # edge MLP hidden layers
                        h1 = {}
                        mlps = ["F", "f"] + (["r"] if l > 0 else [])
                        for nm in mlps:
                            hp = psC.tile([128, 512], f32, tag="h1")
                            nc.tensor.matmul(
                                hp[:, :w], lhsT=W(f"{nm}_W1_{l}"), rhs=mijT[:, :w],
                                start=True, stop=True,
                            )
                            hs = sbh.tile([128, 512], bf16, tag="h1s")
                            nc.scalar.activation(
                                out=hs[:, :w], in_=hp[:, :w], func=Act.Silu,
                                bias=b1col(l, nm),
                            )
                            h1[nm] = hs
                        if l > 0:
                            h1rs = sbh.tile([128, 512], bf16, tag="h1sw")
                            nc.vector.scalar_tensor_tensor(
                                out=h1rs[:, :w], in0=h1["r"][:, :w], scalar=1.0,
                                in1=swc[:, boff : boff + w],
                                op0=Alu.mult, op1=Alu.mult,
                            )

                        for j in range(gw):
                            b = b0 + j
                            first_b = first and j == 0
                            last_b = last_g and j == gw - 1
                            mp = psD.tile([128, 512], f32, tag="mm2")
                            nc.tensor.matmul(
                                mp[:, 0:128],
                                lhsT=h1["f"][:, 128 * j : 128 * (j + 1)],
                                rhs=W(f"f_W2_{l}"), start=True, stop=True,
                            )
                            nc.tensor.matmul(
                                mp[:, 128:129],
                                lhsT=h1["F"][:, 128 * j : 128 * (j + 1)],
                                rhs=W(f"F_W2_{l}")[:, 0:1], start=True, stop=True,
                            )
                            if l > 0:
                                nc.tensor.matmul(
                                    phx[:, 0:128],
                                    lhsT=h1rs[:, 128 * j : 128 * (j + 1)],
                                    rhs=W(f"r_W2_{l}"),
                                    start=first_b, stop=last_b,
                                )
                            # xiacc (PSUM) += I.T @ mij_j
                            nc.tensor.matmul(
                                phx[:, 128:256],
                                lhsT=identb_sb[:],
                                rhs=mij[:, 128 * j : 128 * (j + 1)],
                                start=first_b, stop=last_b,
                            )
                            # g = f_out * (F_raw + bF) | fhat, SBUF bf16
                            ga = sbp.tile([128, 132], bf16, tag="ga")
                            fh = sbp.tile([128, 1], f32, tag="fh")
                            nc.vector.tensor_scalar(
                                out=fh[:, 0:1], in0=mp[:, 128:129],
                                scalar1=bFcols_sb[:, l : l + 1], scalar2=None,
                                op0=Alu.add,
                            )
                            nc.scalar.activation(
                                out=ga[:, 128:129], in_=mp[:, 128:129],
                                func=Act.Identity, bias=bFcols_sb[:, l : l + 1],
                            )
                            nc.vector.tensor_scalar(
                                out=ga[:, 0:128], in0=mp[:, 0:128],
                                scalar1=fh[:, 0:1], scalar2=None,
                                op0=Alu.mult,
                            )
                            # fi_ps[k] += diag(dirsw_k).T @ [g | fhat]
                            for k in range(3):
                                nc.tensor.matmul(
                                    fi_ps[:, 129 * k : 129 * k + 129],
                                    lhsT=dd[:, 384 * (b - cb0) + 128 * k
                                            : 384 * (b - cb0) + 128 * (k + 1)],
                                    rhs=ga[:, 0:129],
                                    start=first_b, stop=last_b,
                                )
                        r0 += gw
                    # ---- tile end folds ----
                    xt = xi_sb[:, 128 * t : 128 * (t + 1)]
                    nc.vector.tensor_tensor(out=xt, in0=xt, in1=phx[:, 128:256],
                                            op=Alu.add)
                    # fi_k += fi_ps_k + b2f * q_k
                    for k in range(3):
                        fslice = fi_sb[:, (3 * t + k) * 128 : (3 * t + k + 1) * 128]
                        nc.vector.scalar_tensor_tensor(
                            out=fslice, in0=fi_ps[:, 129 * k : 129 * k + 128],
                            scalar=1.0,
                            in1=fslice, op0=Alu.mult, op1=Alu.add,
                        )
                        nc.vector.scalar_tensor_tensor(
                            out=fslice, in0=Bb(f"f_{l}"),
                            scalar=fi_ps[:, 129 * k + 128 : 129 * k + 129],
                            in1=fslice, op0=Alu.mult, op1=Alu.add,
                        )
                    if l > 0:
                        # phi = phi_ps + b2r * swsum  (evacuate PSUM)
                        nc.vector.scalar_tensor_tensor(
                            out=phi_sb[:, 128 * t : 128 * (t + 1)],
                            in0=Bb(f"r_{l}"), scalar=swsum_sb[:, t : t + 1],
                            in1=phx[:, 0:128], op0=Alu.mult, op1=Alu.add,
                        )

                # ---- node phase ----
                # transpose xi (now xi_mid) -> xiT
                for t0 in range(0, NT, 4):
                    tn = min(4, NT - t0)
                    ps = psA.tile([128, 512], f32, tag="ps512a")
                    for j in range(tn):
                        nc.tensor.transpose(
                            out=ps[:, 128 * j : 128 * (j + 1)],
                            in_=xi_sb[:, 128 * (t0 + j) : 128 * (t0 + j + 1)],
                            identity=identf_sb[:],
                        )
                    nc.scalar.copy(out=xiT_sb[:, 128 * t0 : 128 * (t0 + tn)],
                                   in_=ps[:, : 128 * tn])
                node_mlp(l, "R", xiT_sb, n_pad, Rout_sb, f"R_{l}")
                node_mlp(l, "u", xiT_sb, n_pad, uout_sb, f"u_{l}")

                # di update (full width)
                def k4(sb, k):
                    a = sb[:]
                    return bass.AP(tensor=a.tensor, offset=a.offset + 128 * k,
                                   ap=[a.ap[0], [384, NT], [1, 128]])
                fi4 = bass.AP(tensor=fi_sb[:].tensor, offset=fi_sb[:].offset,
                              ap=[fi_sb[:].ap[0], [384, NT], [128, 3], [1, 128]])
                di4 = bass.AP(tensor=di_sb[:].tensor, offset=di_sb[:].offset,
                              ap=[di_sb[:].ap[0], [384, NT], [128, 3], [1, 128]])
                R4 = bass.AP(tensor=Rout_sb[:].tensor, offset=Rout_sb[:].offset,
                             ap=[Rout_sb[:].ap[0], [128, NT], [0, 3], [1, 128]])
                if l == 0:
                    nc.vector.tensor_tensor(out=di4, in0=fi4, in1=R4, op=Alu.mult)
                else:
                    phi3 = bass.AP(tensor=phi_sb[:].tensor, offset=phi_sb[:].offset,
                                   ap=[phi_sb[:].ap[0], [128, NT], [1, 128]])
                    t3b = bass.AP(tensor=tmpB_sb[:].tensor, offset=tmpB_sb[:].offset,
                                  ap=[tmpB_sb[:].ap[0], [128, NT], [1, 128]])
                    R3 = bass.AP(tensor=Rout_sb[:].tensor, offset=Rout_sb[:].offset,
                                 ap=[Rout_sb[:].ap[0], [128, NT], [1, 128]])
                    for k in range(3):
                        fk, dk = k4(fi_sb, k), k4(di_sb, k)
                        nc.vector.tensor_tensor(out=t3b, in0=fk, in1=R3, op=Alu.mult)
                        nc.vector.tensor_tensor(out=dk, in0=dk, in1=phi3, op=Alu.mult)
                        nc.vector.tensor_tensor(out=dk, in0=dk, in1=t3b, op=Alu.add)
                # scal = sum_k fi_k * di_k
                s3 = bass.AP(tensor=scal_sb[:].tensor, offset=scal_sb[:].offset,
                             ap=[scal_sb[:].ap[0], [128, NT], [1, 128]])
                t3 = bass.AP(tensor=tmpB_sb[:].tensor, offset=tmpB_sb[:].offset,
                             ap=[tmpB_sb[:].ap[0], [128, NT], [1, 128]])
                nc.vector.tensor_tensor(out=s3, in0=k4(fi_sb, 0), in1=k4(di_sb, 0), op=Alu.mult)
                nc.vector.tensor_tensor(out=t3, in0=k4(fi_sb, 1), in1=k4(di_sb, 1), op=Alu.mult)
                nc.vector.tensor_tensor(out=s3, in0=s3, in1=t3, op=Alu.add)
                nc.vector.tensor_tensor(out=t3, in0=k4(fi_sb, 2), in1=k4(di_sb, 2), op=Alu.mult)
                nc.vector.tensor_tensor(out=s3, in0=s3, in1=t3, op=Alu.add)
                # xi += -u * scal
                nc.vector.scalar_tensor_tensor(
                    out=tmpB_sb[:], in0=uout_sb[:], scalar=-1.0, in1=scal_sb[:],
                    op0=Alu.mult, op1=Alu.mult,
                )
                nc.vector.tensor_tensor(out=xi_sb[:], in0=xi_sb[:], in1=tmpB_sb[:], op=Alu.add)

            # ---- output ----
            nc.sync.dma_start(
                xi_out_d[:].rearrange("(t p) c -> p t c", p=128),
                xi_sb[:].rearrange("p (t c) -> p t c", t=NT),
            )

    nc.compile()
    return nc


# ---------------------------------------------------------------------------
# Entry point
# ---------------------------------------------------------------------------

_CACHE = {}


def kernel(**inputs):
    from concourse.bass_utils import run_bass_kernel_spmd

    meta, per_core = host_prep(inputs)
    wts, widx, bcols, bbc, bidx = pack_weights(inputs)

    key = (meta["NT"], meta["B"], tuple(meta["R"]))
    if key not in _CACHE:
        _CACHE[key] = build_nc(meta, widx, bidx, wts.shape[0], bbc.shape[0])
    nc = _CACHE[key]

    in_maps = make_in_maps(meta, per_core, wts, bcols, bbc)
    res = run_bass_kernel_spmd(nc, in_maps, core_ids=list(range(N_CORES)))

    N = np.asarray(inputs["species"]).shape[0]
    out = np.zeros((N, DIM), np.float32)
    for ci, pc in enumerate(per_core):
        out[pc["perm"]] = res.results[ci]["xi_out"][: pc["n_own"]]
    return out


def make_in_maps(meta, per_core, wts, bcols, bbc):
    import ml_dtypes

    b = lambda x: x.astype(ml_dtypes.bfloat16)
    iota = np.arange(128, dtype=np.float32)[:, None]
    specb_glob = b(meta["specb_glob"])
    in_maps = []
    for pc in per_core:
        in_maps.append(
            dict(
                wts=b(wts), bcols=bcols, bbc=b(bbc),
                rbsw=b(pc["rbsw"]), dsti=pc["dsti_w"],
                ddiag=pc["ddiag"], bFcols=meta["bFcols"],
                swb=b(pc["swb"]), swsum=pc["swsum"],
                specb_own=b(pc["specb_own"]), specb_glob=specb_glob,
                iota128=iota,
            )
        )
    return in_maps
